# revision 22
# baseline (speedup 1.0000x reference)
"""Multi-head self-attention (CogView PB-relax variant) on 8 TRN2 NeuronCores.

Problem: B=2, S=2048, D=1024, H=16 heads, Dh=64.
  q/k/v = hidden @ W{q,k,v}.T + b          (per-head slices)
  scores = (q k^T + attn_bias) / 8 + (1-mask)*(-BIG)
  out    = softmax(scores) @ v             (PB-relax softmax == plain softmax)

Sharding: tensor-parallel over heads. Core c owns heads (2c, 2c+1) for both
batch rows: it reads full hidden, W-row slices [128c:128c+128], bias slice
[h=2c:2c+2], and writes output channels [128c:128(c+1)].

Design (v14, ~1.6x over the 312us v7 baseline):
  - mask-gather: the attention mask kills ~half the k positions; the host
    compacts K/V tokens and bias rows to the unmasked set per batch row
    (padded to a multiple of 256).  Pure indexing on the host; the device
    applies the mask bias to the padded tail.  ~40% less phase-2 work.
  - host repacks every input into the exact per-partition-contiguous layout
    the device consumes (large DMA packets); attention bias travels as fp8.
  - phase 2 per (b,kc): bias lands in PSUM via a PE identity-inject matmul,
    both heads' scores accumulate on top in ONE 2-bank PSUM tile, and ONE
    batched ACT exp(scale*x + maskcol) fuses drain+mask+scale+softmax-exp.
    Nothing latency-critical ever enters the DVE FIFO (its strict FIFO
    behind the epilogue's transpose-waits poisons the pipeline).
  - AV uses a 65th all-ones lhsT column so the softmax denominator falls
    out of the same matmul (ctx row 64).
  - epilogue: DVE drains ctx, one DMA xbar-transpose per head carries both
    the context AND the denominator row, DVE does recip + broadcast-mult.
    No PE transposes.
  - proj chunks and attention blocks are emitted interleaved so the PE
    stays HAM-warm and attention starts ~15us into the kernel.
"""

import math

import numpy as np
import ml_dtypes

import concourse.bass as bass
import concourse.mybir as mybir
import concourse.tile as tile
from concourse import bacc, bass_utils
from concourse.masks import make_identity

F32 = mybir.dt.float32
BF16 = mybir.dt.bfloat16
FP8 = mybir.dt.float8e4
I32 = mybir.dt.int32
Exp = mybir.ActivationFunctionType.Exp

B, S, D = 2, 2048, 1024
NCORES = 8
HPC = 2            # heads per core
OC = HPC * 64      # 128 output channels per core
QB = 512           # q block (free dim of score tiles)
NQB = S // QB      # 4
NDC = D // 128     # 8 contraction chunks
NSB = (B * S) // QB    # 8 q-token chunks for Q projection

MASK_NEG = -30000.0
SCALE = 0.125


def _build_program(nkcs):
    """nkcs = gathered 128-wide k chunks per batch row (each even)."""
    nkt = sum(nkcs)

    nc = bacc.Bacc(
        "TRN2", target_bir_lowering=False, debug=False, num_devices=NCORES
    )
    # host-repacked inputs (all per-partition contiguous)
    hq = nc.dram_tensor("hq", [NSB, 128, NDC, QB], BF16, kind="ExternalInput").ap()
    hgs = [nc.dram_tensor(f"hg{b}", [nkcs[b] // 2, 128, NDC, 256], BF16,
                          kind="ExternalInput").ap() for b in range(B)]
    btgs = [nc.dram_tensor(f"btg{b}", [NQB, 128, HPC, nkcs[b], QB], FP8,
                           kind="ExternalInput").ap() for b in range(B)]
    mkg = nc.dram_tensor("mkg", [128, nkt], I32, kind="ExternalInput").ap()
    ws = [nc.dram_tensor(n, [128, NDC, 128], BF16, kind="ExternalInput").ap()
          for n in ("wq", "wk", "wv")]
    bs = [nc.dram_tensor(n, [OC], F32, kind="ExternalInput").ap()
          for n in ("bq", "bk", "bv")]
    out = nc.dram_tensor("out", [B, NQB, 128, 4, OC], F32, kind="ExternalOutput").ap()

    with tile.TileContext(nc) as tc:
        _attention(tc, out, hq, hgs, btgs, mkg, ws, bs, nkcs)

    nc.compile()
    return nc


def _attention(tc, out, hq, hgs, btgs, mkg, ws, bs, nkcs):
    nc = tc.nc
    nkt = sum(nkcs)
    koff = [0, nkcs[0] * 128]      # kt2 column offset per b
    kbo = [0, nkcs[0]]             # va / mask chunk offset per b

    with tc.tile_pool(name="singles", bufs=1) as singles:
        identb = singles.tile([128, 128], BF16)  # PE bias-inject matmuls
        make_identity(nc, identb)

        # --- weights (wq loaded later, after the first K/V chunks) --------
        wt3 = [singles.tile([128, NDC, 128], BF16, tag=f"wt{i}", name=f"wt{i}")
               for i in range(3)]
        nc.scalar.dma_start(out=wt3[1], in_=ws[1])
        nc.scalar.dma_start(out=wt3[2], in_=ws[2])

        # --- mask -> additive bias column layout [128, nkt] ---------------
        mi = singles.tile([128, nkt], I32)
        nc.scalar.dma_start(out=mi, in_=mkg)
        mf = singles.tile([128, nkt], F32)
        nc.vector.tensor_copy(out=mf, in_=mi)
        mb = singles.tile([128, nkt], F32)
        nc.vector.tensor_scalar(
            out=mb, in0=mf, scalar1=-MASK_NEG, scalar2=MASK_NEG,
            op0=mybir.AluOpType.mult, op1=mybir.AluOpType.add,
        )

        # --- projection bias vectors [128, 1] -----------------------------
        bvec = []
        for i, b_ap in enumerate(bs):
            t = singles.tile([128, 1], F32, tag=f"bvec{i}")
            nc.gpsimd.dma_start(out=t, in_=b_ap.rearrange("(p o) -> p o", o=1))
            bvec.append(t)

        # --- persistent activations (bf16) --------------------------------
        qt2 = singles.tile([128, B * S], BF16, tag="qt2")
        kt2 = singles.tile([128, nkt * 128], BF16, tag="kt2")
        # AV stationary operand: [k-part, kb, head, 64 v-cols + ones + pad]
        va = singles.tile([128, nkt, HPC, 66], BF16, tag="va")
        nc.vector.memset(va, 1.0)   # bakes the ones column; v cols overwritten

        # epilogue staging: 80 partitions (xbar multiple); rows 65-79 are
        # zero filler, memset once
        stage2 = [singles.tile([80, HPC, QB], BF16, tag=f"stage{i}",
                               name=f"stage{i}")
                  for i in range(2)]
        nc.vector.memset(stage2[0], 0.0)
        nc.vector.memset(stage2[1], 0.0)

        with tc.tile_pool(name="h_t", bufs=3) as htp, \
             tc.tile_pool(name="v_t", bufs=6) as vtp, \
             tc.tile_pool(name="b_t", bufs=4) as btp, \
             tc.tile_pool(name="pt", bufs=6) as ptp, \
             tc.tile_pool(name="stage", bufs=2) as stp, \
             tc.tile_pool(name="ot", bufs=3) as otp, \
             tc.tile_pool(name="osb", bufs=2) as osp, \
             tc.tile_pool(name="sc_ps", bufs=3, space="PSUM") as scp, \
             tc.tile_pool(name="ctx_ps", bufs=1, space="PSUM") as cxp:

            vt_tiles = {}

            def kv_chunk(b, g):
                """K+V projection for 256 gathered tokens; fills kt2, vt."""
                hts = htp.tile([128, NDC, 256], BF16, tag="hts", name=f"kv{b}{g}")
                nc.sync.dma_start(out=hts, in_=hgs[b][g])
                # K
                pp = scp.tile([128, 256], F32, tag="sc", name="kps")
                for dc in range(NDC):
                    nc.tensor.matmul(
                        out=pp, lhsT=wt3[1][:, dc, :], rhs=hts[:, dc, :],
                        start=(dc == 0), stop=(dc == NDC - 1))
                dst = kt2[:, koff[b] + g * 256:koff[b] + (g + 1) * 256]
                nc.vector.tensor_scalar_add(out=dst, in0=pp, scalar1=bvec[1])
                # V
                pv = scp.tile([128, 256], F32, tag="sc", name="vps")
                for dc in range(NDC):
                    nc.tensor.matmul(
                        out=pv, lhsT=wt3[2][:, dc, :], rhs=hts[:, dc, :],
                        start=(dc == 0), stop=(dc == NDC - 1))
                vt = vtp.tile([128, 256], BF16, tag="vt", name=f"vt{b}{g}")
                nc.vector.tensor_scalar_add(out=vt, in0=pv, scalar1=bvec[2])
                vt_tiles[(b, g)] = vt

            def v_fin(b, g):
                """Transpose the V chunk and scatter it into va."""
                vt = vt_tiles.pop((b, g))
                vts = vtp.tile([128, 2, 128], BF16, tag="vts", name=f"vts{b}{g}")
                # scalar hwdge queue: keeps the sync queue free for loads
                nc.scalar.dma_start(out=vts, in_=vt, transpose=True)
                for j in range(2):
                    kb = kbo[b] + g * 2 + j
                    for h in range(HPC):
                        nc.vector.tensor_copy(
                            out=va[:, kb, h, 0:64],
                            in_=vts[:, j, h * 64:(h + 1) * 64])

            def q_chunk(qsb):
                """Q projection for 512 tokens (all tokens, ungathered)."""
                hts = htp.tile([128, NDC, QB], BF16, tag="hts", name=f"q{qsb}")
                nc.gpsimd.dma_start(out=hts, in_=hq[qsb])
                pp = scp.tile([128, QB], F32, tag="sc", name="qps")
                for dc in range(NDC):
                    nc.tensor.matmul(
                        out=pp, lhsT=wt3[0][:, dc, :], rhs=hts[:, dc, :],
                        start=(dc == 0), stop=(dc == NDC - 1))
                dst = qt2[:, qsb * QB:(qsb + 1) * QB]
                nc.vector.tensor_scalar_add(out=dst, in0=pp, scalar1=bvec[0])

            def att_block(qb, b, btg_tile):
                """Scores+softmax+AV+epilogue for one (q-block, batch)."""
                nkc = nkcs[b]
                ctx = cxp.tile([65, HPC * QB], F32, tag="ctx", name=f"ctx{qb}{b}")
                for kc in range(nkc):
                    sc = scp.tile([128, HPC * QB], F32, tag="sc", name="sc")
                    for h in range(HPC):
                        nc.tensor.matmul(
                            out=sc[:, h * QB:(h + 1) * QB],
                            lhsT=identb, rhs=btg_tile[:, h, kc, :],
                            start=True, stop=False, skip_group_check=True)
                    for h in range(HPC):
                        nc.tensor.matmul(
                            out=sc[:, h * QB:(h + 1) * QB],
                            lhsT=kt2[h * 64:(h + 1) * 64,
                                     koff[b] + kc * 128:koff[b] + (kc + 1) * 128],
                            rhs=qt2[h * 64:(h + 1) * 64,
                                    b * S + qb * QB:b * S + (qb + 1) * QB],
                            start=False, stop=True,
                            tile_position=(h * 64, 0),
                            skip_group_check=True)
                    pt = ptp.tile([128, HPC, QB], BF16, tag="pt", name="pt")
                    nc.scalar.activation(
                        out=pt.rearrange("p h q -> p (h q)"), in_=sc,
                        func=Exp, bias=mb[:, kbo[b] + kc:kbo[b] + kc + 1],
                        scale=SCALE)
                    for h in range(HPC):
                        nc.tensor.matmul(
                            out=ctx[:, h * QB:(h + 1) * QB],
                            lhsT=va[:, kbo[b] + kc, h, 0:65],
                            rhs=pt[:, h, :],
                            start=(kc == 0), stop=(kc == nkc - 1))
                # ---- epilogue: drain, transpose, normalize, store --------
                # ONE transpose per head carries the 64 v-channels AND the
                # denominator row (stage rows 65-79 are zero filler)
                stage = stage2[(qb * B + b) % 2]
                for h in range(HPC):
                    nc.vector.tensor_copy(
                        out=stage[0:65, h, :], in_=ctx[:, h * QB:(h + 1) * QB])
                ot = otp.tile([128, HPC, 4, 80], BF16, tag="ot", name="ot")
                for h in range(HPC):
                    nc.sync.dma_start(
                        out=ot[:, h, :, :], in_=stage[:, h, :],
                        transpose=True)
                rcp = stp.tile([128, HPC, 4], F32, tag="rcp", name="rcp")
                osb = osp.tile([128, 4, OC], F32, tag="osb", name="osb")
                for h in range(HPC):
                    nc.vector.reciprocal(
                        out=rcp[:, h, :],
                        in_=ot[:, h, :, 64:65].rearrange("p i o -> p (i o)"))
                    nc.vector.tensor_tensor(
                        out=osb[:, :, h * 64:(h + 1) * 64],
                        in0=ot[:, h, :, 0:64],
                        in1=rcp[:, h, :].unsqueeze(2).broadcast_to((128, 4, 64)),
                        op=mybir.AluOpType.mult)
                nc.sync.dma_start(out=out[b, qb], in_=osb)

            # ---- emission schedule -------------------------------------
            def load_btg(b, qb):
                t = btp.tile([128, HPC, nkcs[b], QB], FP8, tag=f"btg{b}",
                             name=f"btg{qb}{b}", bufs=2)
                # two half-programs on different queues: the block can
                # start on the first half, and two DMA program streams
                # keep the engines fed
                half = nkcs[b] // 2
                nc.scalar.dma_start(out=t[:, :, 0:half, :],
                                    in_=btgs[b][qb, :, :, 0:half, :])
                nc.gpsimd.dma_start(out=t[:, :, half:, :],
                                    in_=btgs[b][qb, :, :, half:, :])
                return t

            # block order with btg prefetched ~2 blocks ahead so the PE
            # never waits on a bias DMA stuck behind an epilogue transpose
            blocks = [(qb, b) for qb in range(NQB) for b in range(B)]
            pending = {}

            def prefetch(i):
                if i < len(blocks) and i not in pending:
                    qb, b = blocks[i]
                    pending[i] = load_btg(b, qb)

            nc.scalar.dma_start(out=wt3[0], in_=ws[0])
            for g in range(nkcs[0] // 2):
                kv_chunk(0, g)
            prefetch(0)
            prefetch(1)
            for g in range(nkcs[0] // 2):
                v_fin(0, g)
            for g in range(nkcs[1] // 2):
                kv_chunk(1, g)
            q_chunk(0)
            for i, (qb, b) in enumerate(blocks):
                # Q chunk needed by the *next* block (order 0,4,1,5,2,6,3,7)
                if i + 1 < len(blocks):
                    nqb, nb = blocks[i + 1]
                    q_chunk(nb * NQB + nqb)
                prefetch(i + 2)
                att_block(qb, b, pending.pop(i))
                if i == 0:
                    # b1's V transposes sit behind block-0's exps in the
                    # ACT FIFO, by which time their inputs are long ready
                    for g in range(nkcs[1] // 2):
                        v_fin(1, g)


_CACHE = {}


def _get_program(nkcs):
    if nkcs not in _CACHE:
        _CACHE[nkcs] = _build_program(nkcs)
    return _CACHE[nkcs]


def _prep_inputs(inputs):
    """Host-side prep: sharding, layout packing, gathers, dtype casts."""
    bf = ml_dtypes.bfloat16
    f8 = ml_dtypes.float8_e4m3fn
    hs = np.asarray(inputs["hidden_state"], dtype=np.float32)
    am = np.asarray(inputs["attention_mask"], dtype=np.int32)
    ab = np.asarray(inputs["attention_bias"], dtype=np.float32)
    wts = {k: np.asarray(inputs[k], dtype=np.float32) for k in ("Wq", "Wk", "Wv")}
    vb = {k: np.ascontiguousarray(np.asarray(inputs[k], dtype=np.float32))
          for k in ("bq", "bk", "bv")}

    # gathered k positions per batch row, padded to a multiple of 256
    idx = [np.flatnonzero(am[b]).astype(np.int64) for b in range(B)]
    nkcs = tuple(min(16, 2 * int(math.ceil(max(len(i), 1) / 256.0)))
                 for i in idx)
    kcaps = [n * 128 for n in nkcs]
    nkt = sum(nkcs)

    # device mask columns [128, nkt] (b-major chunk concat)
    mk = np.zeros((nkt, 128), dtype=np.int32)
    off = 0
    for b in range(B):
        n = len(idx[b])
        mkb = np.zeros(kcaps[b], dtype=np.int32)
        mkb[:n] = 1
        mk[off:off + nkcs[b]] = mkb.reshape(nkcs[b], 128)
        off += nkcs[b]
    mkg = np.ascontiguousarray(mk.T).astype(np.int32)

    # hidden^T [D, B*S] once
    hidT = np.ascontiguousarray(hs.reshape(B * S, D).T)  # [D, B*S] f32
    # Q staging: [NSB, 128, NDC, 512]
    hq = np.ascontiguousarray(
        hidT.reshape(NDC, 128, NSB, QB).transpose(2, 1, 0, 3)).astype(bf)
    # gathered K/V staging per b: [nkv_b, 128, NDC, 256]
    hgl = []
    for b in range(B):
        n = len(idx[b])
        hgt = np.zeros((D, kcaps[b]), dtype=np.float32)
        hgt[:, :n] = hidT[:, b * S + idx[b]]
        hgl.append(np.ascontiguousarray(
            hgt.reshape(NDC, 128, nkcs[b] // 2, 256).transpose(2, 1, 0, 3)
        ).astype(bf))

    in_maps = []
    for c in range(NCORES):
        r0, r1 = c * OC, (c + 1) * OC
        m = {
            "hq": hq, "hg0": hgl[0], "hg1": hgl[1], "mkg": mkg,
            "bq": vb["bq"][r0:r1], "bk": vb["bk"][r0:r1],
            "bv": vb["bv"][r0:r1],
        }
        # bias^T gathered per b: [NQB, 128, HPC, nkc_b, QB] fp8
        for b in range(B):
            n = len(idx[b])
            bg = np.zeros((HPC, kcaps[b], S), dtype=np.float32)
            for hh in range(HPC):
                # ab[0, h] is [q, k]; transpose to [k, q], gather k rows
                bg[hh, :n] = ab[0, HPC * c + hh].T[idx[b]]
            m[f"btg{b}"] = np.ascontiguousarray(
                bg.reshape(HPC, nkcs[b], 128, NQB, QB).transpose(3, 2, 0, 1, 4)
            ).astype(f8)
        for nm, key in (("wq", "Wq"), ("wk", "Wk"), ("wv", "Wv")):
            wt = np.ascontiguousarray(wts[key][r0:r1].T)  # [D, OC]
            m[nm] = np.ascontiguousarray(
                wt.reshape(NDC, 128, OC).transpose(1, 0, 2)).astype(bf)
        in_maps.append(m)
    return in_maps, nkcs


def _assemble(res):
    parts = []
    for c in range(NCORES):
        o = np.asarray(res.results[c]["out"])  # [B, NQB, 128, 4, OC]
        parts.append(o.transpose(0, 1, 3, 2, 4).reshape(B, S, OC))
    return np.concatenate(parts, axis=-1)


def kernel(**inputs):
    in_maps, nkcs = _prep_inputs(inputs)
    nc = _get_program(nkcs)
    res = bass_utils.run_bass_kernel_spmd(
        nc, in_maps, core_ids=list(range(NCORES)))
    return _assemble(res)


def run_profiled(inputs, trace=True):
    """test.py helper: returns (output, BassKernelResults)."""
    in_maps, nkcs = _prep_inputs(inputs)
    nc = _get_program(nkcs)
    res = bass_utils.run_bass_kernel_spmd(
        nc, in_maps, core_ids=list(range(NCORES)), trace=trace)
    return _assemble(res), res


# revision 24
# speedup vs baseline: 1.0284x; 1.0284x over previous
"""Multi-head self-attention (CogView PB-relax variant) on 8 TRN2 NeuronCores.

Problem: B=2, S=2048, D=1024, H=16 heads, Dh=64.
  q/k/v = hidden @ W{q,k,v}.T + b          (per-head slices)
  scores = (q k^T + attn_bias) / 8 + (1-mask)*(-BIG)
  out    = softmax(scores) @ v             (PB-relax softmax == plain softmax)

Sharding: tensor-parallel over heads. Core c owns heads (2c, 2c+1) for both
batch rows: it reads full hidden, W-row slices [128c:128c+128], bias slice
[h=2c:2c+2], and writes output channels [128c:128(c+1)].

Design (v14, ~1.6x over the 312us v7 baseline):
  - mask-gather: the attention mask kills ~half the k positions; the host
    compacts K/V tokens and bias rows to the unmasked set per batch row
    (padded to a multiple of 256).  Pure indexing on the host; the device
    applies the mask bias to the padded tail.  ~40% less phase-2 work.
  - host repacks every input into the exact per-partition-contiguous layout
    the device consumes (large DMA packets); attention bias travels as fp8.
  - phase 2 per (b,kc): bias lands in PSUM via a PE identity-inject matmul,
    both heads' scores accumulate on top in ONE 2-bank PSUM tile, and ONE
    batched ACT exp(scale*x + maskcol) fuses drain+mask+scale+softmax-exp.
    Nothing latency-critical ever enters the DVE FIFO (its strict FIFO
    behind the epilogue's transpose-waits poisons the pipeline).
  - AV uses a 65th all-ones lhsT column so the softmax denominator falls
    out of the same matmul (ctx row 64).
  - epilogue: DVE drains ctx, one DMA xbar-transpose per head carries both
    the context AND the denominator row, DVE does recip + broadcast-mult.
    No PE transposes.
  - proj chunks and attention blocks are emitted interleaved so the PE
    stays HAM-warm and attention starts ~15us into the kernel.
"""

import math

import numpy as np
import ml_dtypes

import concourse.bass as bass
import concourse.mybir as mybir
import concourse.tile as tile
from concourse import bacc, bass_utils
from concourse.masks import make_identity

F32 = mybir.dt.float32
BF16 = mybir.dt.bfloat16
FP8 = mybir.dt.float8e4
I32 = mybir.dt.int32
Exp = mybir.ActivationFunctionType.Exp

B, S, D = 2, 2048, 1024
NCORES = 8
HPC = 2            # heads per core
OC = HPC * 64      # 128 output channels per core
QB = 512           # q block (free dim of score tiles)
NQB = S // QB      # 4
NDC = D // 128     # 8 contraction chunks
NSB = (B * S) // QB    # 8 q-token chunks for Q projection

MASK_NEG = -30000.0
SCALE = 0.125

# bf16 Schraudolph exp on the DVE for these mid-block kc positions
# (frees the ACT engine, the steady-state limiter)
DVE_EXP_KCS = (5, 7)
SCH_A = 23.0831       # 0.125 * 128/ln(2)
SCH_B = 16251.0       # 127*128 - sigma
SCH_PAD = 50.0        # masked/pad columns: score is exactly 0 -> tiny denormal


def _build_program(nkcs):
    """nkcs = gathered 128-wide k chunks per batch row (each even)."""
    nkt = sum(nkcs)

    nc = bacc.Bacc(
        "TRN2", target_bir_lowering=False, debug=False, num_devices=NCORES
    )
    # host-repacked inputs (all per-partition contiguous)
    hq = nc.dram_tensor("hq", [NSB, 128, NDC, QB], BF16, kind="ExternalInput").ap()
    hgs = [nc.dram_tensor(f"hg{b}", [nkcs[b] // 2, 128, NDC, 256], BF16,
                          kind="ExternalInput").ap() for b in range(B)]
    btgs = [nc.dram_tensor(f"btg{b}", [NQB, 128, HPC, nkcs[b], QB], FP8,
                           kind="ExternalInput").ap() for b in range(B)]
    mkg = nc.dram_tensor("mkg", [128, nkt], I32, kind="ExternalInput").ap()
    ws = [nc.dram_tensor(n, [128, NDC, 128], BF16, kind="ExternalInput").ap()
          for n in ("wq", "wk", "wv")]
    bs = [nc.dram_tensor(n, [OC], F32, kind="ExternalInput").ap()
          for n in ("bq", "bk", "bv")]
    out = nc.dram_tensor("out", [B, NQB, 128, 4, OC], F32, kind="ExternalOutput").ap()

    with tile.TileContext(nc) as tc:
        _attention(tc, out, hq, hgs, btgs, mkg, ws, bs, nkcs)

    nc.compile()
    return nc


def _attention(tc, out, hq, hgs, btgs, mkg, ws, bs, nkcs):
    nc = tc.nc
    nkt = sum(nkcs)
    koff = [0, nkcs[0] * 128]      # kt2 column offset per b
    kbo = [0, nkcs[0]]             # va / mask chunk offset per b

    with tc.tile_pool(name="singles", bufs=1) as singles:
        identb = singles.tile([128, 128], BF16)  # PE bias-inject matmuls
        make_identity(nc, identb)

        # --- weights (wq loaded later, after the first K/V chunks) --------
        wt3 = [singles.tile([128, NDC, 128], BF16, tag=f"wt{i}", name=f"wt{i}")
               for i in range(3)]
        nc.scalar.dma_start(out=wt3[1], in_=ws[1])
        nc.scalar.dma_start(out=wt3[2], in_=ws[2])

        # --- mask -> additive bias column layout [128, nkt] ---------------
        mi = singles.tile([128, nkt], I32)
        nc.scalar.dma_start(out=mi, in_=mkg)
        mf = singles.tile([128, nkt], F32)
        nc.vector.tensor_copy(out=mf, in_=mi)
        mb = singles.tile([128, nkt], F32)
        nc.vector.tensor_scalar(
            out=mb, in0=mf, scalar1=-MASK_NEG, scalar2=MASK_NEG,
            op0=mybir.AluOpType.mult, op1=mybir.AluOpType.add,
        )
        mb2 = singles.tile([128, nkt], F32)
        nc.vector.tensor_scalar(
            out=mb2, in0=mf, scalar1=SCH_B - SCH_PAD, scalar2=SCH_PAD,
            op0=mybir.AluOpType.mult, op1=mybir.AluOpType.add,
        )

        # --- projection bias vectors [128, 1] -----------------------------
        bvec = []
        for i, b_ap in enumerate(bs):
            t = singles.tile([128, 1], F32, tag=f"bvec{i}")
            nc.gpsimd.dma_start(out=t, in_=b_ap.rearrange("(p o) -> p o", o=1))
            bvec.append(t)

        # --- persistent activations (bf16) --------------------------------
        qt2 = singles.tile([128, B * S], BF16, tag="qt2")
        kt2 = singles.tile([128, nkt * 128], BF16, tag="kt2")
        # AV stationary operand: [k-part, kb, head, 64 v-cols + ones + pad]
        va = singles.tile([128, nkt, HPC, 66], BF16, tag="va")
        nc.vector.memset(va, 1.0)   # bakes the ones column; v cols overwritten

        # epilogue staging: 80 partitions (xbar multiple); rows 65-79 are
        # zero filler, memset once
        stage2 = [singles.tile([80, HPC, QB], BF16, tag=f"stage{i}",
                               name=f"stage{i}")
                  for i in range(2)]
        nc.vector.memset(stage2[0], 0.0)
        nc.vector.memset(stage2[1], 0.0)

        with tc.tile_pool(name="h_t", bufs=3) as htp, \
             tc.tile_pool(name="v_t", bufs=6) as vtp, \
             tc.tile_pool(name="b_t", bufs=4) as btp, \
             tc.tile_pool(name="pt", bufs=6) as ptp, \
             tc.tile_pool(name="stage", bufs=2) as stp, \
             tc.tile_pool(name="ot", bufs=3) as otp, \
             tc.tile_pool(name="osb", bufs=2) as osp, \
             tc.tile_pool(name="sc_ps", bufs=3, space="PSUM") as scp, \
             tc.tile_pool(name="ctx_ps", bufs=1, space="PSUM") as cxp:

            vt_tiles = {}

            def kv_chunk(b, g):
                """K+V projection for 256 gathered tokens; fills kt2, vt."""
                hts = htp.tile([128, NDC, 256], BF16, tag="hts", name=f"kv{b}{g}")
                nc.sync.dma_start(out=hts, in_=hgs[b][g])
                # K
                pp = scp.tile([128, 256], F32, tag="sc", name="kps")
                for dc in range(NDC):
                    nc.tensor.matmul(
                        out=pp, lhsT=wt3[1][:, dc, :], rhs=hts[:, dc, :],
                        start=(dc == 0), stop=(dc == NDC - 1))
                dst = kt2[:, koff[b] + g * 256:koff[b] + (g + 1) * 256]
                nc.vector.tensor_scalar_add(out=dst, in0=pp, scalar1=bvec[1])
                # V
                pv = scp.tile([128, 256], F32, tag="sc", name="vps")
                for dc in range(NDC):
                    nc.tensor.matmul(
                        out=pv, lhsT=wt3[2][:, dc, :], rhs=hts[:, dc, :],
                        start=(dc == 0), stop=(dc == NDC - 1))
                vt = vtp.tile([128, 256], BF16, tag="vt", name=f"vt{b}{g}")
                nc.vector.tensor_scalar_add(out=vt, in0=pv, scalar1=bvec[2])
                vt_tiles[(b, g)] = vt

            def v_fin(b, g):
                """Transpose the V chunk and scatter it into va."""
                vt = vt_tiles.pop((b, g))
                vts = vtp.tile([128, 2, 128], BF16, tag="vts", name=f"vts{b}{g}")
                # scalar hwdge queue: keeps the sync queue free for loads
                nc.scalar.dma_start(out=vts, in_=vt, transpose=True)
                for j in range(2):
                    kb = kbo[b] + g * 2 + j
                    for h in range(HPC):
                        nc.vector.tensor_copy(
                            out=va[:, kb, h, 0:64],
                            in_=vts[:, j, h * 64:(h + 1) * 64])

            def q_chunk(qsb):
                """Q projection for 512 tokens (all tokens, ungathered)."""
                hts = htp.tile([128, NDC, QB], BF16, tag="hts", name=f"q{qsb}")
                nc.gpsimd.dma_start(out=hts, in_=hq[qsb])
                pp = scp.tile([128, QB], F32, tag="sc", name="qps")
                for dc in range(NDC):
                    nc.tensor.matmul(
                        out=pp, lhsT=wt3[0][:, dc, :], rhs=hts[:, dc, :],
                        start=(dc == 0), stop=(dc == NDC - 1))
                dst = qt2[:, qsb * QB:(qsb + 1) * QB]
                nc.vector.tensor_scalar_add(out=dst, in0=pp, scalar1=bvec[0])

            def att_block(qb, b, btg_tile):
                """Scores+softmax+AV+epilogue for one (q-block, batch)."""
                nkc = nkcs[b]
                ctx = cxp.tile([65, HPC * QB], F32, tag="ctx", name=f"ctx{qb}{b}")
                for kc in range(nkc):
                    sc = scp.tile([128, HPC * QB], F32, tag="sc", name="sc")
                    for h in range(HPC):
                        nc.tensor.matmul(
                            out=sc[:, h * QB:(h + 1) * QB],
                            lhsT=identb, rhs=btg_tile[:, h, kc, :],
                            start=True, stop=False, skip_group_check=True)
                    for h in range(HPC):
                        nc.tensor.matmul(
                            out=sc[:, h * QB:(h + 1) * QB],
                            lhsT=kt2[h * 64:(h + 1) * 64,
                                     koff[b] + kc * 128:koff[b] + (kc + 1) * 128],
                            rhs=qt2[h * 64:(h + 1) * 64,
                                    b * S + qb * QB:b * S + (qb + 1) * QB],
                            start=False, stop=True,
                            tile_position=(h * 64, 0),
                            skip_group_check=True)
                    pt = ptp.tile([128, HPC, QB], BF16, tag="pt", name="pt")
                    col = kbo[b] + kc
                    if kc in DVE_EXP_KCS:
                        # bf16 Schraudolph: exp(x) ~ bitcast(i16(A*x + B));
                        # one fused DVE op replaces the ACT exp
                        pti = pt.bitcast(mybir.dt.int16)
                        nc.vector.tensor_scalar(
                            out=pti.rearrange("p h q -> p (h q)"), in0=sc,
                            scalar1=SCH_A, scalar2=mb2[:, col:col + 1],
                            op0=mybir.AluOpType.mult, op1=mybir.AluOpType.add)
                    else:
                        nc.scalar.activation(
                            out=pt.rearrange("p h q -> p (h q)"), in_=sc,
                            func=Exp, bias=mb[:, col:col + 1], scale=SCALE)
                    for h in range(HPC):
                        nc.tensor.matmul(
                            out=ctx[:, h * QB:(h + 1) * QB],
                            lhsT=va[:, kbo[b] + kc, h, 0:65],
                            rhs=pt[:, h, :],
                            start=(kc == 0), stop=(kc == nkc - 1))
                # ---- epilogue: drain, transpose, normalize, store --------
                # ONE transpose per head carries the 64 v-channels AND the
                # denominator row (stage rows 65-79 are zero filler)
                stage = stage2[(qb * B + b) % 2]
                for h in range(HPC):
                    nc.vector.tensor_copy(
                        out=stage[0:65, h, :], in_=ctx[:, h * QB:(h + 1) * QB])
                ot = otp.tile([128, HPC, 4, 80], BF16, tag="ot", name="ot")
                for h in range(HPC):
                    nc.sync.dma_start(
                        out=ot[:, h, :, :], in_=stage[:, h, :],
                        transpose=True)
                rcp = stp.tile([128, HPC, 4], F32, tag="rcp", name="rcp")
                osb = osp.tile([128, 4, OC], F32, tag="osb", name="osb")
                for h in range(HPC):
                    nc.vector.reciprocal(
                        out=rcp[:, h, :],
                        in_=ot[:, h, :, 64:65].rearrange("p i o -> p (i o)"))
                    nc.vector.tensor_tensor(
                        out=osb[:, :, h * 64:(h + 1) * 64],
                        in0=ot[:, h, :, 0:64],
                        in1=rcp[:, h, :].unsqueeze(2).broadcast_to((128, 4, 64)),
                        op=mybir.AluOpType.mult)
                nc.sync.dma_start(out=out[b, qb], in_=osb)

            # ---- emission schedule -------------------------------------
            def load_btg(b, qb):
                t = btp.tile([128, HPC, nkcs[b], QB], FP8, tag=f"btg{b}",
                             name=f"btg{qb}{b}", bufs=2)
                nc.scalar.dma_start(out=t, in_=btgs[b][qb])
                return t

            # block order with btg prefetched ~2 blocks ahead so the PE
            # never waits on a bias DMA stuck behind an epilogue transpose
            blocks = [(qb, b) for qb in range(NQB) for b in range(B)]
            pending = {}

            def prefetch(i):
                if i < len(blocks) and i not in pending:
                    qb, b = blocks[i]
                    pending[i] = load_btg(b, qb)

            # HAM warm-up: keep the PE busy during the startup DMAs so
            # the first real matmuls run at full clock
            warm = scp.tile([128, 128], F32, tag="sc", name="warm")
            for _ in range(48):
                nc.tensor.matmul(out=warm, lhsT=identb, rhs=identb,
                                 start=True, stop=True, skip_group_check=True)
            nc.scalar.dma_start(out=wt3[0], in_=ws[0])
            for g in range(nkcs[0] // 2):
                kv_chunk(0, g)
            prefetch(0)
            prefetch(1)
            for g in range(nkcs[0] // 2):
                v_fin(0, g)
            for g in range(nkcs[1] // 2):
                kv_chunk(1, g)
            q_chunk(0)
            for i, (qb, b) in enumerate(blocks):
                # Q chunk needed by the *next* block (order 0,4,1,5,2,6,3,7)
                if i + 1 < len(blocks):
                    nqb, nb = blocks[i + 1]
                    q_chunk(nb * NQB + nqb)
                prefetch(i + 2)
                att_block(qb, b, pending.pop(i))
                if i == 0:
                    # b1's V transposes sit behind block-0's exps in the
                    # ACT FIFO, by which time their inputs are long ready
                    for g in range(nkcs[1] // 2):
                        v_fin(1, g)


_CACHE = {}


def _get_program(nkcs):
    if nkcs not in _CACHE:
        _CACHE[nkcs] = _build_program(nkcs)
    return _CACHE[nkcs]


def _prep_inputs(inputs):
    """Host-side prep: sharding, layout packing, gathers, dtype casts."""
    bf = ml_dtypes.bfloat16
    f8 = ml_dtypes.float8_e4m3fn
    hs = np.asarray(inputs["hidden_state"], dtype=np.float32)
    am = np.asarray(inputs["attention_mask"], dtype=np.int32)
    ab = np.asarray(inputs["attention_bias"], dtype=np.float32)
    wts = {k: np.asarray(inputs[k], dtype=np.float32) for k in ("Wq", "Wk", "Wv")}
    vb = {k: np.ascontiguousarray(np.asarray(inputs[k], dtype=np.float32))
          for k in ("bq", "bk", "bv")}

    # gathered k positions per batch row, padded to a multiple of 256
    idx = [np.flatnonzero(am[b]).astype(np.int64) for b in range(B)]
    nkcs = tuple(min(16, 2 * int(math.ceil(max(len(i), 1) / 256.0)))
                 for i in idx)
    kcaps = [n * 128 for n in nkcs]
    nkt = sum(nkcs)

    # device mask columns [128, nkt] (b-major chunk concat)
    mk = np.zeros((nkt, 128), dtype=np.int32)
    off = 0
    for b in range(B):
        n = len(idx[b])
        mkb = np.zeros(kcaps[b], dtype=np.int32)
        mkb[:n] = 1
        mk[off:off + nkcs[b]] = mkb.reshape(nkcs[b], 128)
        off += nkcs[b]
    mkg = np.ascontiguousarray(mk.T).astype(np.int32)

    # hidden^T [D, B*S] once
    hidT = np.ascontiguousarray(hs.reshape(B * S, D).T)  # [D, B*S] f32
    # Q staging: [NSB, 128, NDC, 512]
    hq = np.ascontiguousarray(
        hidT.reshape(NDC, 128, NSB, QB).transpose(2, 1, 0, 3)).astype(bf)
    # gathered K/V staging per b: [nkv_b, 128, NDC, 256]
    hgl = []
    for b in range(B):
        n = len(idx[b])
        hgt = np.zeros((D, kcaps[b]), dtype=np.float32)
        hgt[:, :n] = hidT[:, b * S + idx[b]]
        hgl.append(np.ascontiguousarray(
            hgt.reshape(NDC, 128, nkcs[b] // 2, 256).transpose(2, 1, 0, 3)
        ).astype(bf))

    in_maps = []
    for c in range(NCORES):
        r0, r1 = c * OC, (c + 1) * OC
        m = {
            "hq": hq, "hg0": hgl[0], "hg1": hgl[1], "mkg": mkg,
            "bq": vb["bq"][r0:r1], "bk": vb["bk"][r0:r1],
            "bv": vb["bv"][r0:r1],
        }
        # bias^T gathered per b: [NQB, 128, HPC, nkc_b, QB] fp8
        for b in range(B):
            n = len(idx[b])
            bg = np.zeros((HPC, kcaps[b], S), dtype=np.float32)
            for hh in range(HPC):
                # ab[0, h] is [q, k]; transpose to [k, q], gather k rows
                bg[hh, :n] = ab[0, HPC * c + hh].T[idx[b]]
            m[f"btg{b}"] = np.ascontiguousarray(
                bg.reshape(HPC, nkcs[b], 128, NQB, QB).transpose(3, 2, 0, 1, 4)
            ).astype(f8)
        for nm, key in (("wq", "Wq"), ("wk", "Wk"), ("wv", "Wv")):
            wt = np.ascontiguousarray(wts[key][r0:r1].T)  # [D, OC]
            m[nm] = np.ascontiguousarray(
                wt.reshape(NDC, 128, OC).transpose(1, 0, 2)).astype(bf)
        in_maps.append(m)
    return in_maps, nkcs


def _assemble(res):
    parts = []
    for c in range(NCORES):
        o = np.asarray(res.results[c]["out"])  # [B, NQB, 128, 4, OC]
        parts.append(o.transpose(0, 1, 3, 2, 4).reshape(B, S, OC))
    return np.concatenate(parts, axis=-1)


def kernel(**inputs):
    in_maps, nkcs = _prep_inputs(inputs)
    nc = _get_program(nkcs)
    res = bass_utils.run_bass_kernel_spmd(
        nc, in_maps, core_ids=list(range(NCORES)))
    return _assemble(res)


def run_profiled(inputs, trace=True):
    """test.py helper: returns (output, BassKernelResults)."""
    in_maps, nkcs = _prep_inputs(inputs)
    nc = _get_program(nkcs)
    res = bass_utils.run_bass_kernel_spmd(
        nc, in_maps, core_ids=list(range(NCORES)), trace=trace)
    return _assemble(res), res


# revision 25
# speedup vs baseline: 1.0788x; 1.0490x over previous
"""Multi-head self-attention (CogView PB-relax variant) on 8 TRN2 NeuronCores.

Problem: B=2, S=2048, D=1024, H=16 heads, Dh=64.
  q/k/v = hidden @ W{q,k,v}.T + b          (per-head slices)
  scores = (q k^T + attn_bias) / 8 + (1-mask)*(-BIG)
  out    = softmax(scores) @ v             (PB-relax softmax == plain softmax)

Sharding: tensor-parallel over heads. Core c owns heads (2c, 2c+1) for both
batch rows: it reads full hidden, W-row slices [128c:128c+128], bias slice
[h=2c:2c+2], and writes output channels [128c:128(c+1)].

Design (v14, ~1.6x over the 312us v7 baseline):
  - mask-gather: the attention mask kills ~half the k positions; the host
    compacts K/V tokens and bias rows to the unmasked set per batch row
    (padded to a multiple of 256).  Pure indexing on the host; the device
    applies the mask bias to the padded tail.  ~40% less phase-2 work.
  - host repacks every input into the exact per-partition-contiguous layout
    the device consumes (large DMA packets); attention bias travels as fp8.
  - phase 2 per (b,kc): bias lands in PSUM via a PE identity-inject matmul,
    both heads' scores accumulate on top in ONE 2-bank PSUM tile, and ONE
    batched ACT exp(scale*x + maskcol) fuses drain+mask+scale+softmax-exp.
    Nothing latency-critical ever enters the DVE FIFO (its strict FIFO
    behind the epilogue's transpose-waits poisons the pipeline).
  - AV uses a 65th all-ones lhsT column so the softmax denominator falls
    out of the same matmul (ctx row 64).
  - epilogue: DVE drains ctx, one DMA xbar-transpose per head carries both
    the context AND the denominator row, DVE does recip + broadcast-mult.
    No PE transposes.
  - proj chunks and attention blocks are emitted interleaved so the PE
    stays HAM-warm and attention starts ~15us into the kernel.
"""

import math

import numpy as np
import ml_dtypes

import concourse.bass as bass
import concourse.mybir as mybir
import concourse.tile as tile
from concourse import bacc, bass_utils
from concourse.masks import make_identity

F32 = mybir.dt.float32
BF16 = mybir.dt.bfloat16
FP8 = mybir.dt.float8e4
I32 = mybir.dt.int32
Exp = mybir.ActivationFunctionType.Exp

B, S, D = 2, 2048, 1024
NCORES = 8
HPC = 2            # heads per core
OC = HPC * 64      # 128 output channels per core
QB = 512           # q block (free dim of score tiles)
NQB = S // QB      # 4
NDC = D // 128     # 8 contraction chunks
NSB = (B * S) // QB    # 8 q-token chunks for Q projection

MASK_NEG = -30000.0
SCALE = 0.125

# bf16 Schraudolph exp on the DVE for these mid-block kc positions
# (frees the ACT engine, the steady-state limiter)
DVE_EXP_KCS = ()
SCH_A = 23.0831       # 0.125 * 128/ln(2)
SCH_B = 16251.0       # 127*128 - sigma
SCH_PAD = 50.0        # masked/pad columns: score is exactly 0 -> tiny denormal


def _build_program(nkcs):
    """nkcs = gathered 128-wide k chunks per batch row (each even)."""
    nkt = sum(nkcs)

    nc = bacc.Bacc(
        "TRN2", target_bir_lowering=False, debug=False, num_devices=NCORES
    )
    # host-repacked inputs (all per-partition contiguous)
    hq = nc.dram_tensor("hq", [NSB, 128, NDC, QB], BF16, kind="ExternalInput").ap()
    hgs = [nc.dram_tensor(f"hg{b}", [nkcs[b] // 2, 128, NDC, 256], BF16,
                          kind="ExternalInput").ap() for b in range(B)]
    btgs = [nc.dram_tensor(f"btg{b}", [NQB, 128, HPC, nkcs[b], QB], FP8,
                           kind="ExternalInput").ap() for b in range(B)]
    mkg = nc.dram_tensor("mkg", [128, nkt], I32, kind="ExternalInput").ap()
    ws = [nc.dram_tensor(n, [128, NDC, 128], BF16, kind="ExternalInput").ap()
          for n in ("wq", "wk", "wv")]
    bs = [nc.dram_tensor(n, [OC], F32, kind="ExternalInput").ap()
          for n in ("bq", "bk", "bv")]
    out = nc.dram_tensor("out", [B, NQB, 128, 4, OC], F32, kind="ExternalOutput").ap()

    with tile.TileContext(nc) as tc:
        _attention(tc, out, hq, hgs, btgs, mkg, ws, bs, nkcs)

    nc.compile()
    return nc


def _attention(tc, out, hq, hgs, btgs, mkg, ws, bs, nkcs):
    nc = tc.nc
    nkt = sum(nkcs)
    koff = [0, nkcs[0] * 128]      # kt2 column offset per b
    kbo = [0, nkcs[0]]             # va / mask chunk offset per b

    with tc.tile_pool(name="singles", bufs=1) as singles:
        identb = singles.tile([128, 128], BF16)  # PE bias-inject matmuls
        make_identity(nc, identb)

        # --- weights (wq loaded later, after the first K/V chunks) --------
        wt3 = [singles.tile([128, NDC, 128], BF16, tag=f"wt{i}", name=f"wt{i}")
               for i in range(3)]
        nc.scalar.dma_start(out=wt3[1], in_=ws[1])
        nc.scalar.dma_start(out=wt3[2], in_=ws[2])

        # --- mask -> additive bias column layout [128, nkt] ---------------
        mi = singles.tile([128, nkt], I32)
        nc.scalar.dma_start(out=mi, in_=mkg)
        mf = singles.tile([128, nkt], F32)
        nc.vector.tensor_copy(out=mf, in_=mi)
        mb = singles.tile([128, nkt], F32)
        nc.vector.tensor_scalar(
            out=mb, in0=mf, scalar1=-MASK_NEG, scalar2=MASK_NEG,
            op0=mybir.AluOpType.mult, op1=mybir.AluOpType.add,
        )
        mb2 = singles.tile([128, nkt], F32)
        nc.vector.tensor_scalar(
            out=mb2, in0=mf, scalar1=SCH_B - SCH_PAD, scalar2=SCH_PAD,
            op0=mybir.AluOpType.mult, op1=mybir.AluOpType.add,
        )

        # --- projection bias vectors [128, 1] -----------------------------
        bvec = []
        for i, b_ap in enumerate(bs):
            t = singles.tile([128, 1], F32, tag=f"bvec{i}")
            nc.gpsimd.dma_start(out=t, in_=b_ap.rearrange("(p o) -> p o", o=1))
            bvec.append(t)

        # --- persistent activations (bf16) --------------------------------
        qt2 = singles.tile([128, B * S], BF16, tag="qt2")
        kt2 = singles.tile([128, nkt * 128], BF16, tag="kt2")
        # AV stationary operand: [k-part, kb, head, 64 v-cols + ones + pad]
        va = singles.tile([128, nkt, HPC, 66], BF16, tag="va")
        nc.vector.memset(va, 1.0)   # bakes the ones column; v cols overwritten

        # epilogue staging: 80 partitions (xbar multiple); rows 65-79 are
        # zero filler, memset once
        stage2 = [singles.tile([80, HPC, QB], BF16, tag=f"stage{i}",
                               name=f"stage{i}")
                  for i in range(2)]
        nc.vector.memset(stage2[0], 0.0)
        nc.vector.memset(stage2[1], 0.0)

        with tc.tile_pool(name="h_t", bufs=3) as htp, \
             tc.tile_pool(name="v_t", bufs=6) as vtp, \
             tc.tile_pool(name="b_t", bufs=4) as btp, \
             tc.tile_pool(name="pt", bufs=6) as ptp, \
             tc.tile_pool(name="stage", bufs=2) as stp, \
             tc.tile_pool(name="ot", bufs=3) as otp, \
             tc.tile_pool(name="osb", bufs=2) as osp, \
             tc.tile_pool(name="sc_ps", bufs=3, space="PSUM") as scp, \
             tc.tile_pool(name="ctx_ps", bufs=1, space="PSUM") as cxp:

            vt_tiles = {}

            def kv_chunk(b, g):
                """K+V projection for 256 gathered tokens; fills kt2, vt."""
                hts = htp.tile([128, NDC, 256], BF16, tag="hts", name=f"kv{b}{g}")
                nc.sync.dma_start(out=hts, in_=hgs[b][g])
                # K
                pp = scp.tile([128, 256], F32, tag="sc", name="kps")
                for dc in range(NDC):
                    nc.tensor.matmul(
                        out=pp, lhsT=wt3[1][:, dc, :], rhs=hts[:, dc, :],
                        start=(dc == 0), stop=(dc == NDC - 1))
                dst = kt2[:, koff[b] + g * 256:koff[b] + (g + 1) * 256]
                nc.vector.tensor_scalar_add(out=dst, in0=pp, scalar1=bvec[1])
                # V
                pv = scp.tile([128, 256], F32, tag="sc", name="vps")
                for dc in range(NDC):
                    nc.tensor.matmul(
                        out=pv, lhsT=wt3[2][:, dc, :], rhs=hts[:, dc, :],
                        start=(dc == 0), stop=(dc == NDC - 1))
                vt = vtp.tile([128, 256], BF16, tag="vt", name=f"vt{b}{g}")
                nc.vector.tensor_scalar_add(out=vt, in0=pv, scalar1=bvec[2])
                vt_tiles[(b, g)] = vt

            def v_fin(b, g):
                """Transpose the V chunk and scatter it into va."""
                vt = vt_tiles.pop((b, g))
                vts = vtp.tile([128, 2, 128], BF16, tag="vts", name=f"vts{b}{g}")
                # scalar hwdge queue: keeps the sync queue free for loads
                nc.scalar.dma_start(out=vts, in_=vt, transpose=True)
                for j in range(2):
                    kb = kbo[b] + g * 2 + j
                    for h in range(HPC):
                        nc.vector.tensor_copy(
                            out=va[:, kb, h, 0:64],
                            in_=vts[:, j, h * 64:(h + 1) * 64])

            def q_chunk(qsb):
                """Q projection for 512 tokens (all tokens, ungathered)."""
                hts = htp.tile([128, NDC, QB], BF16, tag="hts", name=f"q{qsb}")
                nc.gpsimd.dma_start(out=hts, in_=hq[qsb])
                pp = scp.tile([128, QB], F32, tag="sc", name="qps")
                for dc in range(NDC):
                    nc.tensor.matmul(
                        out=pp, lhsT=wt3[0][:, dc, :], rhs=hts[:, dc, :],
                        start=(dc == 0), stop=(dc == NDC - 1))
                dst = qt2[:, qsb * QB:(qsb + 1) * QB]
                nc.vector.tensor_scalar_add(out=dst, in0=pp, scalar1=bvec[0])

            def att_block(qb, b, btg_tile):
                """Scores+softmax+AV+epilogue for one (q-block, batch)."""
                nkc = nkcs[b]
                ctx = cxp.tile([65, HPC * QB], F32, tag="ctx", name=f"ctx{qb}{b}")
                for kc in range(nkc):
                    sc = scp.tile([128, HPC * QB], F32, tag="sc", name="sc")
                    for h in range(HPC):
                        nc.tensor.matmul(
                            out=sc[:, h * QB:(h + 1) * QB],
                            lhsT=identb, rhs=btg_tile[:, h, kc, :],
                            start=True, stop=False, skip_group_check=True)
                    for h in range(HPC):
                        nc.tensor.matmul(
                            out=sc[:, h * QB:(h + 1) * QB],
                            lhsT=kt2[h * 64:(h + 1) * 64,
                                     koff[b] + kc * 128:koff[b] + (kc + 1) * 128],
                            rhs=qt2[h * 64:(h + 1) * 64,
                                    b * S + qb * QB:b * S + (qb + 1) * QB],
                            start=False, stop=True,
                            tile_position=(h * 64, 0),
                            skip_group_check=True)
                    pt = ptp.tile([128, HPC, QB], BF16, tag="pt", name="pt")
                    col = kbo[b] + kc
                    if kc in DVE_EXP_KCS:
                        # bf16 Schraudolph: exp(x) ~ bitcast(i16(A*x + B));
                        # one fused DVE op replaces the ACT exp
                        pti = pt.bitcast(mybir.dt.int16)
                        nc.vector.tensor_scalar(
                            out=pti.rearrange("p h q -> p (h q)"), in0=sc,
                            scalar1=SCH_A, scalar2=mb2[:, col:col + 1],
                            op0=mybir.AluOpType.mult, op1=mybir.AluOpType.add)
                    else:
                        nc.scalar.activation(
                            out=pt.rearrange("p h q -> p (h q)"), in_=sc,
                            func=Exp, bias=mb[:, col:col + 1], scale=SCALE)
                    for h in range(HPC):
                        nc.tensor.matmul(
                            out=ctx[:, h * QB:(h + 1) * QB],
                            lhsT=va[:, kbo[b] + kc, h, 0:65],
                            rhs=pt[:, h, :],
                            start=(kc == 0), stop=(kc == nkc - 1))
                # ---- epilogue: drain, transpose, normalize, store --------
                # ONE transpose per head carries the 64 v-channels AND the
                # denominator row (stage rows 65-79 are zero filler)
                stage = stage2[(qb * B + b) % 2]
                for h in range(HPC):
                    nc.vector.tensor_copy(
                        out=stage[0:65, h, :], in_=ctx[:, h * QB:(h + 1) * QB])
                ot = otp.tile([128, HPC, 4, 80], BF16, tag="ot", name="ot")
                for h in range(HPC):
                    nc.sync.dma_start(
                        out=ot[:, h, :, :], in_=stage[:, h, :],
                        transpose=True)
                rcp = stp.tile([128, HPC, 4], F32, tag="rcp", name="rcp")
                osb = osp.tile([128, 4, OC], F32, tag="osb", name="osb")
                for h in range(HPC):
                    nc.vector.reciprocal(
                        out=rcp[:, h, :],
                        in_=ot[:, h, :, 64:65].rearrange("p i o -> p (i o)"))
                    nc.vector.tensor_tensor(
                        out=osb[:, :, h * 64:(h + 1) * 64],
                        in0=ot[:, h, :, 0:64],
                        in1=rcp[:, h, :].unsqueeze(2).broadcast_to((128, 4, 64)),
                        op=mybir.AluOpType.mult)
                nc.sync.dma_start(out=out[b, qb], in_=osb)

            # ---- emission schedule -------------------------------------
            def load_btg(b, qb):
                t = btp.tile([128, HPC, nkcs[b], QB], FP8, tag=f"btg{b}",
                             name=f"btg{qb}{b}", bufs=2)
                nc.scalar.dma_start(out=t, in_=btgs[b][qb])
                return t

            # block order with btg prefetched ~2 blocks ahead so the PE
            # never waits on a bias DMA stuck behind an epilogue transpose
            blocks = [(qb, b) for qb in range(NQB) for b in range(B)]
            pending = {}

            def prefetch(i):
                if i < len(blocks) and i not in pending:
                    qb, b = blocks[i]
                    pending[i] = load_btg(b, qb)

            # HAM warm-up: keep the PE busy during the startup DMAs so
            # the first real matmuls run at full clock
            warm = scp.tile([128, 128], F32, tag="sc", name="warm")
            for _ in range(48):
                nc.tensor.matmul(out=warm, lhsT=identb, rhs=identb,
                                 start=True, stop=True, skip_group_check=True)
            nc.scalar.dma_start(out=wt3[0], in_=ws[0])
            for g in range(nkcs[0] // 2):
                kv_chunk(0, g)
            prefetch(0)
            prefetch(1)
            for g in range(nkcs[0] // 2):
                v_fin(0, g)
            for g in range(nkcs[1] // 2):
                kv_chunk(1, g)
            q_chunk(0)
            for i, (qb, b) in enumerate(blocks):
                # Q chunk needed by the *next* block (order 0,4,1,5,2,6,3,7)
                if i + 1 < len(blocks):
                    nqb, nb = blocks[i + 1]
                    q_chunk(nb * NQB + nqb)
                prefetch(i + 2)
                att_block(qb, b, pending.pop(i))
                if i == 0:
                    # b1's V transposes sit behind block-0's exps in the
                    # ACT FIFO, by which time their inputs are long ready
                    for g in range(nkcs[1] // 2):
                        v_fin(1, g)


_CACHE = {}


def _get_program(nkcs):
    if nkcs not in _CACHE:
        _CACHE[nkcs] = _build_program(nkcs)
    return _CACHE[nkcs]


def _prep_inputs(inputs):
    """Host-side prep: sharding, layout packing, gathers, dtype casts."""
    bf = ml_dtypes.bfloat16
    f8 = ml_dtypes.float8_e4m3fn
    hs = np.asarray(inputs["hidden_state"], dtype=np.float32)
    am = np.asarray(inputs["attention_mask"], dtype=np.int32)
    ab = np.asarray(inputs["attention_bias"], dtype=np.float32)
    wts = {k: np.asarray(inputs[k], dtype=np.float32) for k in ("Wq", "Wk", "Wv")}
    vb = {k: np.ascontiguousarray(np.asarray(inputs[k], dtype=np.float32))
          for k in ("bq", "bk", "bv")}

    # gathered k positions per batch row, padded to a multiple of 256
    idx = [np.flatnonzero(am[b]).astype(np.int64) for b in range(B)]
    nkcs = tuple(min(16, 2 * int(math.ceil(max(len(i), 1) / 256.0)))
                 for i in idx)
    kcaps = [n * 128 for n in nkcs]
    nkt = sum(nkcs)

    # device mask columns [128, nkt] (b-major chunk concat)
    mk = np.zeros((nkt, 128), dtype=np.int32)
    off = 0
    for b in range(B):
        n = len(idx[b])
        mkb = np.zeros(kcaps[b], dtype=np.int32)
        mkb[:n] = 1
        mk[off:off + nkcs[b]] = mkb.reshape(nkcs[b], 128)
        off += nkcs[b]
    mkg = np.ascontiguousarray(mk.T).astype(np.int32)

    # hidden^T [D, B*S] once
    hidT = np.ascontiguousarray(hs.reshape(B * S, D).T)  # [D, B*S] f32
    # Q staging: [NSB, 128, NDC, 512]
    hq = np.ascontiguousarray(
        hidT.reshape(NDC, 128, NSB, QB).transpose(2, 1, 0, 3)).astype(bf)
    # gathered K/V staging per b: [nkv_b, 128, NDC, 256]
    hgl = []
    for b in range(B):
        n = len(idx[b])
        hgt = np.zeros((D, kcaps[b]), dtype=np.float32)
        hgt[:, :n] = hidT[:, b * S + idx[b]]
        hgl.append(np.ascontiguousarray(
            hgt.reshape(NDC, 128, nkcs[b] // 2, 256).transpose(2, 1, 0, 3)
        ).astype(bf))

    in_maps = []
    for c in range(NCORES):
        r0, r1 = c * OC, (c + 1) * OC
        m = {
            "hq": hq, "hg0": hgl[0], "hg1": hgl[1], "mkg": mkg,
            "bq": vb["bq"][r0:r1], "bk": vb["bk"][r0:r1],
            "bv": vb["bv"][r0:r1],
        }
        # bias^T gathered per b: [NQB, 128, HPC, nkc_b, QB] fp8
        for b in range(B):
            n = len(idx[b])
            bg = np.zeros((HPC, kcaps[b], S), dtype=np.float32)
            for hh in range(HPC):
                # ab[0, h] is [q, k]; transpose to [k, q], gather k rows
                bg[hh, :n] = ab[0, HPC * c + hh].T[idx[b]]
            m[f"btg{b}"] = np.ascontiguousarray(
                bg.reshape(HPC, nkcs[b], 128, NQB, QB).transpose(3, 2, 0, 1, 4)
            ).astype(f8)
        for nm, key in (("wq", "Wq"), ("wk", "Wk"), ("wv", "Wv")):
            wt = np.ascontiguousarray(wts[key][r0:r1].T)  # [D, OC]
            m[nm] = np.ascontiguousarray(
                wt.reshape(NDC, 128, OC).transpose(1, 0, 2)).astype(bf)
        in_maps.append(m)
    return in_maps, nkcs


def _assemble(res):
    parts = []
    for c in range(NCORES):
        o = np.asarray(res.results[c]["out"])  # [B, NQB, 128, 4, OC]
        parts.append(o.transpose(0, 1, 3, 2, 4).reshape(B, S, OC))
    return np.concatenate(parts, axis=-1)


def kernel(**inputs):
    in_maps, nkcs = _prep_inputs(inputs)
    nc = _get_program(nkcs)
    res = bass_utils.run_bass_kernel_spmd(
        nc, in_maps, core_ids=list(range(NCORES)))
    return _assemble(res)


def run_profiled(inputs, trace=True):
    """test.py helper: returns (output, BassKernelResults)."""
    in_maps, nkcs = _prep_inputs(inputs)
    nc = _get_program(nkcs)
    res = bass_utils.run_bass_kernel_spmd(
        nc, in_maps, core_ids=list(range(NCORES)), trace=trace)
    return _assemble(res), res
